# revision 20
# baseline (speedup 1.0000x reference)
"""Trainium2 Bass kernel for CausalGraphAttention (N=8192, F=256), 8-core SPMD.

Math (per reference):
  h      = x @ W                               [N, F]
  e[i,j] = leaky_relu(h[i]@a1 + h[j]@a2, 0.2)
           + (cs[j] - cs[i]) * cw[i,j],   cs = x @ c
  e      = where(adj, e, -9e15);  att = softmax(e, axis=1);  out = att @ h

Device strategy (1D row-parallel, transposed score layout):
  - Each core owns a 1024-row block of the score matrix. All score tiles are
    computed TRANSPOSED: eT[j, i] with j on partitions, i on the free dim, so
    the final contraction over j maps directly onto the tensor engine
    (lhsT = exp(eT) subtile [j,128i], rhs = [h | 1 | 1] tile [j,258]) and
    the softmax denominator falls out of the ones columns of the rhs for free.
  - Graph inputs are host-preprocessed into a single fp16 matrix
    cwm[i,j] = adj ? max(cw, 6.2e-5) : -1  (sign encodes the adjacency mask).
    On device a fused custom DVE op computes
      w = (cs[j] - cs[i]) * cwm + min(1000*cwm, 0)
    so non-edges get e ~= -1000 and exp(e) = 0 exactly (matching the
    reference's exp(-9e15) = 0).
  - s_src/s_dst/cs are folded into matmuls: W_aug = [W@a2 | c | W] gives
    per-j scalars during the h pass; replicated-weight matmuls give the
    per-i row vectors broadcast across all 128 partitions.
  - leaky_relu(ss+sd) alternates between ScalarE (Prelu) and a second fused
    DVE op to balance engine load; exp runs on ScalarE over chunk pairs.
  - Normalization (1/rowsum) is applied to the 1024x256 output block only.
"""

import numpy as np

import concourse.mybir as mybir
import concourse.tile as tile
from concourse import bacc
from concourse import dve_ops as _dops
from concourse.bass_utils import run_bass_kernel_spmd
from concourse.dve_ops import DveOp
from concourse.dve_spec import C0, C1, Spec, Src0, Src1, Zero, _has_src1, lower, minn
from concourse.dve_uop import DveOpSpec

dt = mybir.dt
AF = mybir.ActivationFunctionType
ALU = mybir.AluOpType

N = 8192
F = 256
NCORES = 8
RPC = N // NCORES          # rows per core (i range)
NJT = N // 128             # j tiles of 128
NSUB = RPC // 128          # i subtiles of 128
ALPHA = 0.2
MASK_BIG = 1000.0
HSTRIDE = F + 4            # h tile layout: [h(256) | ones(2) | pad(2)], 8B-aligned
NMM = F + 2                # matmul rhs width: [h | 1 | 1] (even for full-rate streaming)


def _register_dve_op(name, spec):
    for op in _dops.OPS:
        if op.name == name:
            return op
    opcode = _dops._CUSTOM_DVE_ROW_BASE + len(_dops.OPS)
    assert opcode < 0x20
    _dops._SUB_OPCODE_FOR_NAME[name] = opcode
    shas = {}
    for ver in ("v3", "v4"):
        s = DveOpSpec(name=name, opcode=opcode, uops=lower(spec, ver=ver),
                      rd1_en=_has_src1(spec))
        shas[ver] = s.sha(ver)
    op = DveOp(name, spec, subdim=False, uops_sha=shas)
    _dops.OPS.append(op)
    _dops.CUSTOM_DVE_SPECS[name] = op.spec
    return op


# w = (in1 + cs_j) * cwm + min(BIG*cwm, 0)
W_OP = _register_dve_op("CGA_W_FUSED", Spec(
    body=(Src1 + C0) * Src0 + minn(Src0 * C1, Zero),
    reference=lambda in0, in1, s0, s1: (in1 + s0) * in0 + np.minimum(in0 * s1, 0.0)))

# e = leaky_relu_0.2(in0 + sd_j) + in1  (leaky via u + min(u,0)*(-0.8))
_U = Src0 + C0
E_OP = _register_dve_op("CGA_E_LRELU", Spec(
    body=(_U + minn(_U, Zero) * C1) + Src1,
    reference=lambda in0, in1, s0, s1:
        (lambda u: u + np.minimum(u, 0.0) * s1 + in1)(in0 + s0)))


def build_program():
    nc = bacc.Bacc("TRN2", target_bir_lowering=False, debug=False,
                   num_devices=NCORES)

    xT = nc.declare_dram_parameter("xT", [F, N], dt.float16, isOutput=False)
    xTown = nc.declare_dram_parameter("xTown", [F, RPC], dt.float16, isOutput=False)
    Waug = nc.declare_dram_parameter("Waug", [F, F + 2], dt.float16, isOutput=False)
    WA1rep = nc.declare_dram_parameter("WA1rep", [F, 128], dt.float16, isOutput=False)
    WCnegrep = nc.declare_dram_parameter("WCnegrep", [F, 128], dt.float16, isOutput=False)
    cwmT = nc.declare_dram_parameter("cwmT", [N, RPC], dt.float16, isOutput=False)
    out_d = nc.declare_dram_parameter("out", [RPC, F], dt.float32, isOutput=True)

    with tile.TileContext(nc) as tc:
        with (
            tc.tile_pool(name="persist", bufs=1) as persist,
            # main-loop pool is allocated BEFORE the setup pools so that its
            # SBUF range does not overlap the released setup pool (a stack
            # overlap would serialize the whole main loop behind setup).
            tc.tile_pool(name="main", bufs=2) as main_pool,
            tc.tile_pool(name="tail", bufs=2) as tailp,
        ):
            # --- persistent tiles ---
            h_all = persist.tile([128, NJT * HSTRIDE], dt.bfloat16, tag="h_all")
            scols = persist.tile([128, 2 * NJT], dt.float32, tag="scols")
            ss_repl = persist.tile([128, RPC], dt.float16, tag="ss_repl")
            negcs_repl = persist.tile([128, RPC], dt.float16, tag="negcs_repl")
            waug_sb = persist.tile([128, 2, F + 2], dt.float16, tag="waug")
            wa1_sb = persist.tile([128, 2, 128], dt.float16, tag="wa1")
            wcn_sb = persist.tile([128, 2, 128], dt.float16, tag="wcn")
            xtown_sb = persist.tile([128, 2, RPC], dt.float16, tag="xtown")

            nc.sync.dma_start(out=xtown_sb[:], in_=xTown.ap().rearrange("(b p) f -> p b f", p=128))
            nc.sync.dma_start(out=wa1_sb[:], in_=WA1rep.ap().rearrange("(b p) f -> p b f", p=128))
            nc.sync.dma_start(out=wcn_sb[:], in_=WCnegrep.ap().rearrange("(b p) f -> p b f", p=128))
            nc.sync.dma_start(out=waug_sb[:], in_=Waug.ap().rearrange("(b p) f -> p b f", p=128))

            cw_src = cwmT.ap().rearrange("(c p) i -> p c i", p=128)
            cw_tiles = {}

            exp_bias = persist.tile([128, 1], dt.float32, tag="exp_bias")
            nc.vector.memset(exp_bias[:], -4.0)

            # ones column of every h tile
            ones_ap = h_all[:].rearrange("p (t c) -> p t c", c=HSTRIDE)[:, :, F:F + 2]
            nc.vector.memset(ones_ap, 1.0)

            h_view = h_all[:].rearrange("p (t c) -> p t c", c=HSTRIDE)
            sc_view = scols[:].rearrange("p (t c) -> p t c", c=2)

            # --- setup phase ---
            with (
                tc.tile_pool(name="xt_pool", bufs=1) as xt_pool,
            ):
                xt_sb = xt_pool.tile([128, 2, N], dt.float16, tag="xt")
                xt_src = xT.ap().rearrange("(b p) f -> p b f", p=128)
                for blk in range(2):
                    sl = slice(blk * (N // 8), (blk + 1) * (N // 8))
                    nc.sync.dma_start(out=xt_sb[:, :, sl], in_=xt_src[:, :, sl])
                # prefetch the first causal-weight chunks between the xT blocks
                # so the elementwise pipeline can start during setup
                for jw in range(3):
                    cw_pre = main_pool.tile([128, 2, RPC], dt.float16, tag="cw",
                                            bufs=6, name=f"cw_pre{jw}")
                    nc.sync.dma_start(out=cw_pre[:], in_=cw_src[:, 2 * jw:2 * jw + 2, :])
                    cw_tiles[jw] = cw_pre
                for blk in range(2, 8):
                    sl = slice(blk * (N // 8), (blk + 1) * (N // 8))
                    nc.sync.dma_start(out=xt_sb[:, :, sl], in_=xt_src[:, :, sl])

                # replicated s_src[i] and -cs[i] across all partitions
                with tc.tile_pool(name="psum_s", bufs=2, space="PSUM") as psum_s:
                    for (w_sb, dest) in ((wcn_sb, negcs_repl), (wa1_sb, ss_repl)):
                        for half in range(RPC // 512):
                            ps = psum_s.tile([128, 512], dt.float32, tag="ps_s")
                            for kh in range(2):
                                nc.tensor.matmul(
                                    ps[:], lhsT=w_sb[:, kh, :],
                                    rhs=xtown_sb[:, kh, half * 512:(half + 1) * 512],
                                    start=(kh == 0), stop=(kh == 1))
                            # negcs (needed first, by the fused causal op) on DVE,
                            # ss (needed by Prelu) on ScalarE
                            if dest is negcs_repl:
                                nc.vector.tensor_copy(dest[:, half * 512:(half + 1) * 512], ps[:])
                            else:
                                nc.scalar.copy(dest[:, half * 512:(half + 1) * 512], ps[:])

                # express pass: sd/cs scalars for the first 8 chunks via tiny
                # N=2 matmuls, so the main-loop elementwise stream can start
                # long before the batched h-pass copies land
                with tc.tile_pool(name="psum_x", bufs=1, space="PSUM") as psum_x:
                    psx = psum_x.tile([128, 16], dt.float32, tag="ps_x")
                    for jt in range(8):
                        for kh in range(2):
                            nc.tensor.matmul(
                                psx[:, 2 * jt:2 * jt + 2],
                                lhsT=xt_sb[:, kh, jt * 128:(jt + 1) * 128],
                                rhs=waug_sb[:, kh, 0:2],
                                start=(kh == 0), stop=(kh == 1))
                    nc.vector.tensor_copy(scols[:, 0:16], psx[:])

                # h pass, 4 tiles per PSUM batch: h_aug = [sd | cs | h]
                with tc.tile_pool(name="psum_h", bufs=2, space="PSUM") as psum_h:
                    for g in range(NJT // 4):
                        ps = psum_h.tile([128, 4, 512], dt.float32, tag="ps_h")
                        for t in range(4):
                            jt = 4 * g + t
                            for kh in range(2):
                                nc.tensor.matmul(
                                    ps[:, t, 0:F + 2],
                                    lhsT=xt_sb[:, kh, jt * 128:(jt + 1) * 128],
                                    rhs=waug_sb[:, kh, :],
                                    start=(kh == 0), stop=(kh == 1))
                        if g % 2 == 0:
                            nc.scalar.copy(h_view[:, 4 * g:4 * g + 4, 0:F], ps[:, :, 2:F + 2])
                        else:
                            nc.vector.tensor_copy(h_view[:, 4 * g:4 * g + 4, 0:F], ps[:, :, 2:F + 2])
                        if g >= 2:
                            # (gpsimd cannot read PSUM; tiny copy on DVE;
                            # g<2 scalars already written by the express pass)
                            nc.vector.tensor_copy(sc_view[:, 4 * g:4 * g + 4, :], ps[:, :, 0:2])

            # --- main loop ---
            with tc.tile_pool(name="psum_o", bufs=1, space="PSUM") as psum_o:
                out_ps = [psum_o.tile([128, NMM], dt.float32, tag=f"out{s}",
                                      name=f"out_ps{s}")
                          for s in range(NSUB)]

                for jq in range(NJT // 4):
                  e_quad = main_pool.tile([128, 4, RPC], dt.float16, tag="e", bufs=3)
                  p_quad = main_pool.tile([128, 4, RPC], dt.float16, tag="p", bufs=3)
                  for pp in range(2):
                    jp = 2 * jq + pp
                    e_pair = e_quad[:, 2 * pp:2 * pp + 2, :]
                    w_pair = main_pool.tile([128, 2, RPC], dt.float16, tag="w", bufs=4)
                    t1_pair = main_pool.tile([128, 2, RPC], dt.float16, tag="t1", bufs=4)
                    fused = (jp % 5 == 4)  # ~6/64 of jt halves via E_LRELU
                    for par in range(2):
                        jt = 2 * jp + par
                        sd_col = scols[:, 2 * jt:2 * jt + 1]
                        cs_col = scols[:, 2 * jt + 1:2 * jt + 2]

                        jw, half = divmod(jt, 2)
                        if jw in cw_tiles:
                            cw_t = cw_tiles[jw]
                        else:
                            cw_t = main_pool.tile([128, 2, RPC], dt.float16,
                                                  tag="cw", bufs=6, name="cw_t")
                            nc.sync.dma_start(out=cw_t[:],
                                              in_=cw_src[:, 2 * jw:2 * jw + 2, :])
                            cw_tiles[jw] = cw_t
                        cw_ap = cw_t[:, half, :]

                        # w = (cs_j - cs_i)*cwm + min(BIG*cwm, 0)
                        nc.vector._custom_dve(W_OP, out=w_pair[:, par, :], in0=cw_ap,
                                              in1=negcs_repl[:], s0=cs_col,
                                              s1=MASK_BIG)
                        if fused and par == 1:
                            # fused leaky+add on VectorE (balances ScalarE load)
                            nc.vector._custom_dve(E_OP, out=e_pair[:, par, :],
                                                  in0=ss_repl[:], in1=w_pair[:, par, :],
                                                  s0=sd_col, s1=-0.8)
                        else:
                            # t1 = leaky(ss+sd) on ScalarE
                            nc.scalar.activation(t1_pair[:, par, :], ss_repl[:], AF.Prelu,
                                                 bias=sd_col, scale=1.0, alpha=ALPHA)
                    if fused:
                        nc.vector.tensor_tensor(out=e_pair[:, 0, :],
                                                in0=w_pair[:, 0, :],
                                                in1=t1_pair[:, 0, :], op=ALU.add)
                    else:
                        # one paired e-add for both halves
                        nc.vector.tensor_tensor(out=e_pair[:], in0=w_pair[:],
                                                in1=t1_pair[:], op=ALU.add)

                  # p = exp(e - 4) over the quad (0 for masked pairs; the
                  # constant shift keeps p in fp16 range and cancels in the
                  # softmax ratio since the ones-column sum is shifted too)
                  nc.scalar.activation(p_quad[:], e_quad[:], AF.Exp, bias=exp_bias[:])

                  # out[i, :] += p^T @ [h | 1]
                  for qpar in range(4):
                      jt = 4 * jq + qpar
                      for s in range(NSUB):
                          nc.tensor.matmul(
                              out_ps[s][:],
                              lhsT=p_quad[:, qpar, s * 128:(s + 1) * 128],
                              rhs=h_view[:, jt, 0:NMM],
                              start=(jt == 0), stop=(jt == NJT - 1))

                # --- tail: normalize and write out ---
                for s in range(NSUB):
                    rec = tailp.tile([128, 1], dt.float32, tag="rec", bufs=4)
                    nc.vector.reciprocal(rec[:], out_ps[s][:, F:F + 1])
                    o_sb = tailp.tile([128, F], dt.float32, tag="osb", bufs=4)
                    if s % 2 == 0:
                        nc.vector.tensor_scalar(out=o_sb[:], in0=out_ps[s][:, 0:F],
                                                scalar1=rec[:], scalar2=None,
                                                op0=ALU.mult)
                    else:
                        nc.scalar.activation(o_sb[:], out_ps[s][:, 0:F], AF.Copy,
                                             scale=rec[:])
                    nc.sync.dma_start(out=out_d.ap()[s * 128:(s + 1) * 128, :],
                                      in_=o_sb[:])

    nc.compile()
    return nc


_CACHED_NC = None


def _get_program():
    global _CACHED_NC
    if _CACHED_NC is None:
        _CACHED_NC = build_program()
    return _CACHED_NC


def _host_prep(x, adj, causal_weights, W, a1, a2, c):
    x = np.asarray(x, dtype=np.float32)
    adj = np.asarray(adj)
    cw = np.asarray(causal_weights, dtype=np.float32)
    W = np.asarray(W, dtype=np.float32)
    a1 = np.asarray(a1, dtype=np.float32)
    a2 = np.asarray(a2, dtype=np.float32)
    c = np.asarray(c, dtype=np.float32)

    wa1 = W @ a1
    wa2 = W @ a2
    waug = np.concatenate([wa2[:, None], c[:, None], W], axis=1).astype(np.float16)
    wa1rep = np.repeat(wa1[:, None], 128, axis=1).astype(np.float16)
    wcnegrep = np.repeat(-c[:, None], 128, axis=1).astype(np.float16)
    xt16 = np.ascontiguousarray(x.T).astype(np.float16)

    # sign-encoded mask: positive -> edge weight, -1 -> non-edge
    cwm = np.where(adj > 0, np.maximum(cw, 6.2e-5), -1.0).astype(np.float16)

    in_maps = []
    for k in range(NCORES):
        r0, r1 = k * RPC, (k + 1) * RPC
        in_maps.append({
            "xT": xt16,
            "xTown": np.ascontiguousarray(xt16[:, r0:r1]),
            "Waug": waug,
            "WA1rep": wa1rep,
            "WCnegrep": wcnegrep,
            "cwmT": np.ascontiguousarray(cwm[r0:r1, :].T),
        })
    return in_maps


def kernel(x, adj, causal_weights, W, a1, a2, c, _trace=False, _trace_kwargs=None):
    nc = _get_program()
    in_maps = _host_prep(x, adj, causal_weights, W, a1, a2, c)
    kw = {}
    if _trace:
        kw["trace"] = True
        kw.update(_trace_kwargs or {})
    res = run_bass_kernel_spmd(nc, in_maps, list(range(NCORES)), **kw)
    out = np.concatenate([res.results[k]["out"] for k in range(NCORES)], axis=0)
    if _trace:
        return out, res
    return out


# revision 21
# speedup vs baseline: 1.0129x; 1.0129x over previous
"""Trainium2 Bass kernel for CausalGraphAttention (N=8192, F=256), 8-core SPMD.

Math (per reference):
  h      = x @ W                               [N, F]
  e[i,j] = leaky_relu(h[i]@a1 + h[j]@a2, 0.2)
           + (cs[j] - cs[i]) * cw[i,j],   cs = x @ c
  e      = where(adj, e, -9e15);  att = softmax(e, axis=1);  out = att @ h

Device strategy (1D row-parallel, transposed score layout):
  - Each core owns a 1024-row block of the score matrix. All score tiles are
    computed TRANSPOSED: eT[j, i] with j on partitions, i on the free dim, so
    the final contraction over j maps directly onto the tensor engine
    (lhsT = exp(eT) subtile [j,128i], rhs = [h | 1 | 1] tile [j,258]) and
    the softmax denominator falls out of the ones columns of the rhs for free.
  - Graph inputs are host-preprocessed into a single fp16 matrix
    cwm[i,j] = adj ? max(cw, 6.2e-5) : -1  (sign encodes the adjacency mask).
    On device a fused custom DVE op computes
      w = (cs[j] - cs[i]) * cwm + min(1000*cwm, 0)
    so non-edges get e ~= -1000 and exp(e) = 0 exactly (matching the
    reference's exp(-9e15) = 0).
  - s_src/s_dst/cs are folded into matmuls: W_aug = [W@a2 | c | W] gives
    per-j scalars during the h pass; replicated-weight matmuls give the
    per-i row vectors broadcast across all 128 partitions.
  - leaky_relu(ss+sd) alternates between ScalarE (Prelu) and a second fused
    DVE op to balance engine load; exp runs on ScalarE over chunk pairs.
  - Normalization (1/rowsum) is applied to the 1024x256 output block only.
"""

import numpy as np

import concourse.mybir as mybir
import concourse.tile as tile
from concourse import bacc
from concourse import dve_ops as _dops
from concourse.bass_utils import run_bass_kernel_spmd
from concourse.dve_ops import DveOp
from concourse.dve_spec import C0, C1, Spec, Src0, Src1, Zero, _has_src1, lower, minn
from concourse.dve_uop import DveOpSpec

dt = mybir.dt
AF = mybir.ActivationFunctionType
ALU = mybir.AluOpType

N = 8192
F = 256
NCORES = 8
RPC = N // NCORES          # rows per core (i range)
NJT = N // 128             # j tiles of 128
NSUB = RPC // 128          # i subtiles of 128
ALPHA = 0.2
MASK_BIG = 1000.0
HSTRIDE = F + 4            # h tile layout: [h(256) | ones(2) | pad(2)], 8B-aligned
NMM = F + 2                # matmul rhs width: [h | 1 | 1] (even for full-rate streaming)


def _register_dve_op(name, spec):
    for op in _dops.OPS:
        if op.name == name:
            return op
    opcode = _dops._CUSTOM_DVE_ROW_BASE + len(_dops.OPS)
    assert opcode < 0x20
    _dops._SUB_OPCODE_FOR_NAME[name] = opcode
    shas = {}
    for ver in ("v3", "v4"):
        s = DveOpSpec(name=name, opcode=opcode, uops=lower(spec, ver=ver),
                      rd1_en=_has_src1(spec))
        shas[ver] = s.sha(ver)
    op = DveOp(name, spec, subdim=False, uops_sha=shas)
    _dops.OPS.append(op)
    _dops.CUSTOM_DVE_SPECS[name] = op.spec
    return op


# w = (in1 + cs_j) * cwm + min(BIG*cwm, 0)
W_OP = _register_dve_op("CGA_W_FUSED", Spec(
    body=(Src1 + C0) * Src0 + minn(Src0 * C1, Zero),
    reference=lambda in0, in1, s0, s1: (in1 + s0) * in0 + np.minimum(in0 * s1, 0.0)))

# e = leaky_relu_0.2(in0 + sd_j) + in1  (leaky via u + min(u,0)*(-0.8))
_U = Src0 + C0
E_OP = _register_dve_op("CGA_E_LRELU", Spec(
    body=(_U + minn(_U, Zero) * C1) + Src1,
    reference=lambda in0, in1, s0, s1:
        (lambda u: u + np.minimum(u, 0.0) * s1 + in1)(in0 + s0)))


def build_program():
    nc = bacc.Bacc("TRN2", target_bir_lowering=False, debug=False,
                   num_devices=NCORES)

    xT = nc.declare_dram_parameter("xT", [F, N], dt.float16, isOutput=False)
    xTown = nc.declare_dram_parameter("xTown", [F, RPC], dt.float16, isOutput=False)
    Waug = nc.declare_dram_parameter("Waug", [F, F + 2], dt.float16, isOutput=False)
    WA1rep = nc.declare_dram_parameter("WA1rep", [F, 128], dt.float16, isOutput=False)
    WCnegrep = nc.declare_dram_parameter("WCnegrep", [F, 128], dt.float16, isOutput=False)
    cwmT = nc.declare_dram_parameter("cwmT", [N, RPC], dt.float16, isOutput=False)
    out_d = nc.declare_dram_parameter("out", [RPC, F], dt.float32, isOutput=True)

    with tile.TileContext(nc) as tc:
        with (
            tc.tile_pool(name="persist", bufs=1) as persist,
            # main-loop pool is allocated BEFORE the setup pools so that its
            # SBUF range does not overlap the released setup pool (a stack
            # overlap would serialize the whole main loop behind setup).
            tc.tile_pool(name="main", bufs=2) as main_pool,
            tc.tile_pool(name="tail", bufs=2) as tailp,
        ):
            # --- persistent tiles ---
            h_all = persist.tile([128, NJT * HSTRIDE], dt.bfloat16, tag="h_all")
            scols = persist.tile([128, 2 * NJT], dt.float32, tag="scols")
            ss_repl = persist.tile([128, RPC], dt.float16, tag="ss_repl")
            negcs_repl = persist.tile([128, RPC], dt.float16, tag="negcs_repl")
            waug_sb = persist.tile([128, 2, F + 2], dt.float16, tag="waug")
            wa1_sb = persist.tile([128, 2, 128], dt.float16, tag="wa1")
            wcn_sb = persist.tile([128, 2, 128], dt.float16, tag="wcn")
            xtown_sb = persist.tile([128, 2, RPC], dt.float16, tag="xtown")

            nc.sync.dma_start(out=xtown_sb[:], in_=xTown.ap().rearrange("(b p) f -> p b f", p=128))
            nc.sync.dma_start(out=wa1_sb[:], in_=WA1rep.ap().rearrange("(b p) f -> p b f", p=128))
            nc.sync.dma_start(out=wcn_sb[:], in_=WCnegrep.ap().rearrange("(b p) f -> p b f", p=128))
            nc.sync.dma_start(out=waug_sb[:], in_=Waug.ap().rearrange("(b p) f -> p b f", p=128))

            cw_src = cwmT.ap().rearrange("(c p) i -> p c i", p=128)
            cw_tiles = {}

            exp_bias = persist.tile([128, 1], dt.float32, tag="exp_bias")
            nc.vector.memset(exp_bias[:], -4.0)

            # ones column of every h tile
            ones_ap = h_all[:].rearrange("p (t c) -> p t c", c=HSTRIDE)[:, :, F:F + 2]
            nc.vector.memset(ones_ap, 1.0)

            h_view = h_all[:].rearrange("p (t c) -> p t c", c=HSTRIDE)
            sc_view = scols[:].rearrange("p (t c) -> p t c", c=2)

            # --- setup phase ---
            with (
                tc.tile_pool(name="xt_pool", bufs=1) as xt_pool,
            ):
                xt_sb = xt_pool.tile([128, 2, N], dt.float16, tag="xt")
                xt_src = xT.ap().rearrange("(b p) f -> p b f", p=128)
                for blk in range(2):
                    sl = slice(blk * (N // 8), (blk + 1) * (N // 8))
                    nc.sync.dma_start(out=xt_sb[:, :, sl], in_=xt_src[:, :, sl])
                # prefetch the first causal-weight chunks between the xT blocks
                # so the elementwise pipeline can start during setup
                for jw in range(3):
                    cw_pre = main_pool.tile([128, 2, RPC], dt.float16, tag="cw",
                                            bufs=8, name=f"cw_pre{jw}")
                    nc.sync.dma_start(out=cw_pre[:], in_=cw_src[:, 2 * jw:2 * jw + 2, :])
                    cw_tiles[jw] = cw_pre
                for blk in range(2, 8):
                    sl = slice(blk * (N // 8), (blk + 1) * (N // 8))
                    nc.sync.dma_start(out=xt_sb[:, :, sl], in_=xt_src[:, :, sl])

                # replicated s_src[i] and -cs[i] across all partitions
                with tc.tile_pool(name="psum_s", bufs=2, space="PSUM") as psum_s:
                    for (w_sb, dest) in ((wcn_sb, negcs_repl), (wa1_sb, ss_repl)):
                        for half in range(RPC // 512):
                            ps = psum_s.tile([128, 512], dt.float32, tag="ps_s")
                            for kh in range(2):
                                nc.tensor.matmul(
                                    ps[:], lhsT=w_sb[:, kh, :],
                                    rhs=xtown_sb[:, kh, half * 512:(half + 1) * 512],
                                    start=(kh == 0), stop=(kh == 1))
                            # negcs (needed first, by the fused causal op) on DVE,
                            # ss (needed by Prelu) on ScalarE
                            if dest is negcs_repl:
                                nc.vector.tensor_copy(dest[:, half * 512:(half + 1) * 512], ps[:])
                            else:
                                nc.scalar.copy(dest[:, half * 512:(half + 1) * 512], ps[:])

                # h pass, 4 tiles per PSUM batch: h_aug = [sd | cs | h]
                with tc.tile_pool(name="psum_h", bufs=2, space="PSUM") as psum_h:
                    for g in range(NJT // 4):
                        ps = psum_h.tile([128, 4, 512], dt.float32, tag="ps_h")
                        for t in range(4):
                            jt = 4 * g + t
                            for kh in range(2):
                                nc.tensor.matmul(
                                    ps[:, t, 0:F + 2],
                                    lhsT=xt_sb[:, kh, jt * 128:(jt + 1) * 128],
                                    rhs=waug_sb[:, kh, :],
                                    start=(kh == 0), stop=(kh == 1))
                        if g % 2 == 0:
                            nc.scalar.copy(h_view[:, 4 * g:4 * g + 4, 0:F], ps[:, :, 2:F + 2])
                        else:
                            nc.vector.tensor_copy(h_view[:, 4 * g:4 * g + 4, 0:F], ps[:, :, 2:F + 2])
                        # (gpsimd cannot read PSUM; keep this tiny copy on DVE)
                        nc.vector.tensor_copy(sc_view[:, 4 * g:4 * g + 4, :], ps[:, :, 0:2])

            # --- main loop ---
            with tc.tile_pool(name="psum_o", bufs=1, space="PSUM") as psum_o:
                out_ps = [psum_o.tile([128, NMM], dt.float32, tag=f"out{s}",
                                      name=f"out_ps{s}")
                          for s in range(NSUB)]

                for jp in range(NJT // 2):
                    e_pair = main_pool.tile([128, 2, RPC], dt.float16, tag="e", bufs=4)
                    p_pair = main_pool.tile([128, 2, RPC], dt.float16, tag="p", bufs=4)
                    w_pair = main_pool.tile([128, 2, RPC], dt.float16, tag="w", bufs=4)
                    t1_pair = main_pool.tile([128, 2, RPC], dt.float16, tag="t1", bufs=4)
                    fused = (jp % 4 == 3)  # every 8th jt pair-half via E_LRELU
                    for par in range(2):
                        jt = 2 * jp + par
                        sd_col = scols[:, 2 * jt:2 * jt + 1]
                        cs_col = scols[:, 2 * jt + 1:2 * jt + 2]

                        jw, half = divmod(jt, 2)
                        if jw in cw_tiles:
                            cw_t = cw_tiles[jw]
                        else:
                            cw_t = main_pool.tile([128, 2, RPC], dt.float16,
                                                  tag="cw", bufs=8, name="cw_t")
                            nc.sync.dma_start(out=cw_t[:],
                                              in_=cw_src[:, 2 * jw:2 * jw + 2, :])
                            cw_tiles[jw] = cw_t
                        cw_ap = cw_t[:, half, :]

                        # w = (cs_j - cs_i)*cwm + min(BIG*cwm, 0)
                        nc.vector._custom_dve(W_OP, out=w_pair[:, par, :], in0=cw_ap,
                                              in1=negcs_repl[:], s0=cs_col,
                                              s1=MASK_BIG)
                        if fused and par == 1:
                            # fused leaky+add on VectorE (balances ScalarE load)
                            nc.vector._custom_dve(E_OP, out=e_pair[:, par, :],
                                                  in0=ss_repl[:], in1=w_pair[:, par, :],
                                                  s0=sd_col, s1=-0.8)
                        else:
                            # t1 = leaky(ss+sd) on ScalarE
                            nc.scalar.activation(t1_pair[:, par, :], ss_repl[:], AF.Prelu,
                                                 bias=sd_col, scale=1.0, alpha=ALPHA)
                    if fused:
                        nc.vector.tensor_tensor(out=e_pair[:, 0, :],
                                                in0=w_pair[:, 0, :],
                                                in1=t1_pair[:, 0, :], op=ALU.add)
                    else:
                        # one paired e-add for both halves
                        nc.vector.tensor_tensor(out=e_pair[:], in0=w_pair[:],
                                                in1=t1_pair[:], op=ALU.add)

                    # p = exp(e - 4) over the pair (0 for masked pairs; the
                    # constant shift keeps p in fp16 range and cancels in the
                    # softmax ratio since the ones-column sum is shifted too)
                    nc.scalar.activation(p_pair[:], e_pair[:], AF.Exp, bias=exp_bias[:])

                    # out[i, :] += p^T @ [h | 1]
                    for par in range(2):
                        jt = 2 * jp + par
                        for s in range(NSUB):
                            nc.tensor.matmul(
                                out_ps[s][:],
                                lhsT=p_pair[:, par, s * 128:(s + 1) * 128],
                                rhs=h_view[:, jt, 0:NMM],
                                start=(jt == 0), stop=(jt == NJT - 1))

                # --- tail: normalize and write out ---
                for s in range(NSUB):
                    rec = tailp.tile([128, 1], dt.float32, tag="rec", bufs=4)
                    nc.vector.reciprocal(rec[:], out_ps[s][:, F:F + 1])
                    o_sb = tailp.tile([128, F], dt.float32, tag="osb", bufs=4)
                    if s % 2 == 0:
                        nc.vector.tensor_scalar(out=o_sb[:], in0=out_ps[s][:, 0:F],
                                                scalar1=rec[:], scalar2=None,
                                                op0=ALU.mult)
                    else:
                        nc.scalar.activation(o_sb[:], out_ps[s][:, 0:F], AF.Copy,
                                             scale=rec[:])
                    nc.sync.dma_start(out=out_d.ap()[s * 128:(s + 1) * 128, :],
                                      in_=o_sb[:])

    nc.compile()
    return nc


_CACHED_NC = None


def _get_program():
    global _CACHED_NC
    if _CACHED_NC is None:
        _CACHED_NC = build_program()
    return _CACHED_NC


def _host_prep(x, adj, causal_weights, W, a1, a2, c):
    x = np.asarray(x, dtype=np.float32)
    adj = np.asarray(adj)
    cw = np.asarray(causal_weights, dtype=np.float32)
    W = np.asarray(W, dtype=np.float32)
    a1 = np.asarray(a1, dtype=np.float32)
    a2 = np.asarray(a2, dtype=np.float32)
    c = np.asarray(c, dtype=np.float32)

    wa1 = W @ a1
    wa2 = W @ a2
    waug = np.concatenate([wa2[:, None], c[:, None], W], axis=1).astype(np.float16)
    wa1rep = np.repeat(wa1[:, None], 128, axis=1).astype(np.float16)
    wcnegrep = np.repeat(-c[:, None], 128, axis=1).astype(np.float16)
    xt16 = np.ascontiguousarray(x.T).astype(np.float16)

    # sign-encoded mask: positive -> edge weight, -1 -> non-edge
    cwm = np.where(adj > 0, np.maximum(cw, 6.2e-5), -1.0).astype(np.float16)

    in_maps = []
    for k in range(NCORES):
        r0, r1 = k * RPC, (k + 1) * RPC
        in_maps.append({
            "xT": xt16,
            "xTown": np.ascontiguousarray(xt16[:, r0:r1]),
            "Waug": waug,
            "WA1rep": wa1rep,
            "WCnegrep": wcnegrep,
            "cwmT": np.ascontiguousarray(cwm[r0:r1, :].T),
        })
    return in_maps


def kernel(x, adj, causal_weights, W, a1, a2, c, _trace=False, _trace_kwargs=None):
    nc = _get_program()
    in_maps = _host_prep(x, adj, causal_weights, W, a1, a2, c)
    kw = {}
    if _trace:
        kw["trace"] = True
        kw.update(_trace_kwargs or {})
    res = run_bass_kernel_spmd(nc, in_maps, list(range(NCORES)), **kw)
    out = np.concatenate([res.results[k]["out"] for k in range(NCORES)], axis=0)
    if _trace:
        return out, res
    return out
